# revision 41
# baseline (speedup 1.0000x reference)
"""HODLR matvec kernel for 8 TRN2 NeuronCores (Bass/Tile), v2.

Sharding: node axis split into 8 contiguous slices of 32768 nodes.

Per core (all tensor-engine matmuls fp8 DoubleRow, K=256):
  projection  x is the STATIONARY ([128,2,64] per 256-node chunk-pair),
              u the moving ([128,2,256]) -> psum t^T tiles [64 batch, 256].
              Pass C covers levels 0-3 accumulated over each 16384-node L3
              block; pass F covers levels 4-7 per 1024-node L7 block.
  collective  after pass C, levels 0-2 partials are sender-masked and
              ReduceScattered (CCE add) so each core receives its combined
              sibling coefficients; consumed only by the last expansion pass.
  tree        L7 t-tiles summed pairwise (DVE) up to L4; tree outputs are
              PE-transposed (small) into block-structured fp8 stationaries.
  expansion   DR matmuls, moving ut [128,2,512] per 512-node group; groups
              paired into [128,512] psums via tile_position=(0,64).
              Levels 4-7 pass first (local), levels 0-3 pass last (needs
              the collective). corr returned fp8, scaled by USCALE^2.

DMA: one HWDGE FIFO (nc.sync) streams uc -> uf -> ut23 -> ut01 in 1MB
chunks so input bandwidth is saturated from t=0 and every compute phase
is gated only by its own bytes. Host computes diag*x in fp32 and adds
corr/USCALE^2.
"""

import os
import sys

sys.path.insert(0, "/opt/trn_rl_repo")

import numpy as np
import ml_dtypes

BF16 = ml_dtypes.bfloat16
FP8 = ml_dtypes.float8_e4m3

B = 64
N = 262144
NCORES = 8
M = N // NCORES          # 32768 nodes per core
R = 64
DEPTH = 8
CP = M // 256            # 128 chunk-pairs (256 nodes each)
NB7 = M // 1024          # 32 L7 blocks
NG = M // 512            # 64 expansion groups
USCALE = 64.0

_cached = {}


def _build_bass():
    import concourse.bacc as bacc
    import concourse.tile as tile
    import concourse.mybir as mybir
    from contextlib import ExitStack

    BF = mybir.dt.bfloat16
    F8 = mybir.dt.float8e4
    F32 = mybir.dt.float32
    ADD = mybir.AluOpType.add
    MULT = mybir.AluOpType.mult
    DR = mybir.MatmulPerfMode.DoubleRow

    nc = bacc.Bacc(
        "TRN2",
        target_bir_lowering=False,
        debug=False,
        enable_asserts=False,
        num_devices=NCORES,
    )

    xs_d = nc.dram_tensor("xs", [128, CP, 2, B], F8, kind="ExternalInput").ap()
    uc_d = nc.dram_tensor("uc", [128, CP, 2, 256], F8, kind="ExternalInput").ap()
    uf_d = nc.dram_tensor("uf", [128, CP, 2, 256], F8, kind="ExternalInput").ap()
    ut23_d = nc.dram_tensor("ut23", [128, 2, M], F8, kind="ExternalInput").ap()
    ut01_d = nc.dram_tensor("ut01", [128, 2, M], F8, kind="ExternalInput").ap()
    msk_d = nc.dram_tensor("msk", [B, 8, 192], BF, kind="ExternalInput").ap()
    id_d = nc.dram_tensor("ident", [B, B], BF, kind="ExternalInput").ap()
    # corr viewed as [B, pair, half, 512]: pair p = groups (2p, 2p+1)
    corr_d = nc.dram_tensor(
        "corr", [B, NG // 2, 2, 512], F8, kind="ExternalOutput"
    ).ap()

    with tile.TileContext(nc) as tc, ExitStack() as ctx:
        const = ctx.enter_context(tc.tile_pool(name="const", bufs=1))
        up = ctx.enter_context(tc.tile_pool(name="up", bufs=3))
        utp = ctx.enter_context(tc.tile_pool(name="utp", bufs=4))
        pc = ctx.enter_context(tc.tile_pool(name="pc", bufs=1, space="PSUM"))
        pf = ctx.enter_context(tc.tile_pool(name="pf", bufs=2, space="PSUM"))
        ep = ctx.enter_context(tc.tile_pool(name="ep", bufs=5, space="PSUM"))
        treep = ctx.enter_context(tc.tile_pool(name="treep", bufs=1))
        statp = ctx.enter_context(tc.tile_pool(name="statp", bufs=1))
        stgp = ctx.enter_context(tc.tile_pool(name="stgp", bufs=1))
        tpp = ctx.enter_context(tc.tile_pool(name="tpp", bufs=1))
        y23p = ctx.enter_context(tc.tile_pool(name="y23p", bufs=1))
        yop = ctx.enter_context(tc.tile_pool(name="yop", bufs=1))
        dram = ctx.enter_context(tc.tile_pool(name="dram", bufs=1, space="DRAM"))

        # ---- small constants (SWDGE path, parallel to the main FIFO) ----
        xs = const.tile([128, CP, 2, B], F8, tag="xs")
        for q in range(4):
            nc.gpsimd.dma_start(
                xs[:, 32 * q : 32 * (q + 1)], xs_d[:, 32 * q : 32 * (q + 1)]
            )
        ident = const.tile([B, B], BF, tag="ident")
        nc.gpsimd.dma_start(ident[:], id_d[:])
        msk = const.tile([B, 8, 192], BF, tag="msk")
        nc.gpsimd.dma_start(msk[:], msk_d[:])

        def vcopy(i, out, in_):
            if i % 2 == 0:
                nc.vector.tensor_copy(out, in_)
            else:
                nc.scalar.copy(out, in_)

        def vadd(i, out, a, b):
            (nc.vector if i % 2 == 0 else nc.gpsimd).tensor_tensor(
                out, a, b, op=ADD
            )

        # ---------------- pass C: projection levels 0-3 ----------------
        # t^T psum [64, 256] per L3 block (cols l*64+r, l in 0..3)
        psC2 = pc.tile([B, 2, 256], F32, tag="psC")
        psC = [psC2[:, 0], psC2[:, 1]]
        # pieces of (start_j, count): small first pieces cut startup latency
        PIECES = [(0, 8), (8, 8)] + [(16 * t, 16) for t in range(1, 8)]
        for t, (j0, jn) in enumerate(PIECES):
            ucT = up.tile([128, jn, 2, 256], F8, tag="u_in", name=f"uc{t}")
            nc.sync.dma_start(ucT[:], uc_d[:, j0 : j0 + jn])
            for jj in range(jn):
                j = j0 + jj
                nc.tensor.matmul(
                    psC[j // 64][:],
                    xs[:, j],
                    ucT[:, jj],
                    start=(j % 64 == 0),
                    stop=(j % 64 == 63),
                    perf_mode=DR,
                )

        lvl012 = treep.tile([B, 192], BF, tag="lvl012")
        nc.vector.tensor_copy(lvl012[:], psC[0][:, 0:192])
        nc.vector.tensor_tensor(lvl012[:], psC[1][:, 0:192], lvl012[:], op=ADD)
        l3t = treep.tile([B, 2, 64], BF, tag="l3t")
        nc.vector.tensor_copy(l3t[:, 0], psC[0][:, 192:256])
        nc.vector.tensor_copy(l3t[:, 1], psC[1][:, 192:256])
        b_in = dram.tile([B, 192], BF, tag="b_in")
        b_out = dram.tile([8, B, 192], BF, tag="b_out", addr_space="Shared")
        nc.gpsimd.dma_start(b_in[:], lvl012[:])
        nc.gpsimd.collective_compute(
            "AllGather",
            mybir.AluOpType.bypass,
            replica_groups=[list(range(NCORES))],
            ins=[b_in.opt()],
            outs=[b_out.opt()],
        )

        # ---- collective: levels 0-2 sender-masked ReduceScatter ----
        # ---------------- pass F: projection levels 4-7 ----------------
        # Tree adds and SF23 transposes are emitted inside the loop as soon
        # as their dependencies complete, so they overlap pass F on the
        # vector/scalar engines and expansion-23 can start immediately.
        # SF23 [128, 64, 64] fp8:
        #   col 2*m7   : b45[m7//4] = (l4T[(m7//8)^1] | l5T[(m7//4)^1])
        #   col 2*m7+1 : b67[m7]    = (l6T[(m7//2)^1] | l7T[m7^1])
        # SF01 [128, 4, 64]: col 2b : (l0T | l1T);  col 2b+1 : (l2T | l3T[b^1])
        SF23 = statp.tile([128, 64, B], F8, tag="SF23")
        SF01 = statp.tile([128, 4, B], F8, tag="SF01")

        stg_tiles = {}

        def stage_pair(name, srcA, srcB, idx):
            stg = stgp.tile([B, 128], BF, tag=f"stg_{name}", name=f"tr_{name}")
            vcopy(idx, stg[:, 0:64], srcA)
            vcopy(idx + 1, stg[:, 64:128], srcB)
            stg_tiles[name] = stg

        def do_transpose(name, dests, idx):
            tp = tpp.tile([128, B], BF, tag=f"tp_{name}", name=f"tp_{name}")
            nc.scalar.dma_start(tp[:], stg_tiles[name][:], transpose=True)
            for k, (dtile, dcol) in enumerate(dests):
                vcopy(idx + k, dtile[:, dcol], tp[:])

        # block index after which each transpose's inputs exist
        rdy45 = {
            m5: max(8 * ((m5 // 2) ^ 1) + 7, 4 * (m5 ^ 1) + 3)
            for m5 in range(8)
        }
        rdy67 = {
            m7: max(2 * ((m7 // 2) ^ 1) + 1, m7 ^ 1) for m7 in range(NB7)
        }
        Tf, S1, S2, S3 = [], [], [], []
        emitted = set()

        def emit_ready(thresh):
            for m5 in range(8):
                if rdy45[m5] <= thresh and ("b45", m5) not in emitted:
                    emitted.add(("b45", m5))
                    do_transpose(
                        f"b45_{m5}",
                        [(SF23, 2 * mm) for mm in range(4 * m5, 4 * m5 + 4)],
                        m5,
                    )
            for m7 in range(NB7):
                if rdy67[m7] <= thresh and ("b67", m7) not in emitted:
                    emitted.add(("b67", m7))
                    do_transpose(f"b67_{m7}", [(SF23, 2 * m7 + 1)], m7)

        ps = None
        for t, (j0, jn) in enumerate(PIECES):
            ufT = up.tile([128, jn, 2, 256], F8, tag="u_in", name=f"uf{t}")
            nc.sync.dma_start(ufT[:], uf_d[:, j0 : j0 + jn])
            for jj in range(jn):
                j = j0 + jj
                m = j // 4
                if j % 4 == 0:
                    ps = pf.tile([B, 256], F32, tag="psF", name=f"psF{m}")
                nc.tensor.matmul(
                    ps[:],
                    xs[:, j],
                    ufT[:, jj],
                    start=(j % 4 == 0),
                    stop=(j % 4 == 3),
                    perf_mode=DR,
                )
                if j % 4 != 3:
                    continue
                g = treep.tile([B, 256], F8, tag=f"Tf{m}")
                vcopy(m, g[:], ps[:])
                Tf.append(g)
                if m % 2 == 1:
                    s1 = treep.tile([B, 192], BF, tag=f"S1_{m // 2}")
                    vadd(m, s1[:], Tf[m - 1][:, 0:192], Tf[m][:, 0:192])
                    S1.append(s1)
                if m % 4 == 3:
                    s2 = treep.tile([B, 128], BF, tag=f"S2_{m // 4}")
                    vadd(
                        m + 1,
                        s2[:],
                        S1[m // 2 - 1][:, 0:128],
                        S1[m // 2][:, 0:128],
                    )
                    S2.append(s2)
                if m % 8 == 7:
                    s3 = treep.tile([B, 64], BF, tag=f"S3_{m // 8}")
                    vadd(
                        m,
                        s3[:],
                        S2[m // 4 - 1][:, 0:64],
                        S2[m // 4][:, 0:64],
                    )
                    S3.append(s3)
                for m5 in range(8):
                    if rdy45[m5] == m:
                        stage_pair(
                            f"b45_{m5}",
                            S3[(m5 // 2) ^ 1][:, 0:64],
                            S2[m5 ^ 1][:, 64:128],
                            m5,
                        )
                for m7 in range(NB7):
                    if rdy67[m7] == m:
                        stage_pair(
                            f"b67_{m7}",
                            S1[(m7 // 2) ^ 1][:, 128:192],
                            Tf[m7 ^ 1][:, 192:256],
                            m7,
                        )
        # ---------------- expansion ----------------
        SEG = 4096  # nodes per ut tile (one 1MB DMA)
        GPS = SEG // 512  # 16 groups per segment
        y23 = []
        for s in range(M // SEG):
            utT = utp.tile([128, 2, SEG], F8, tag="ut_in", name=f"ut23_{s}")
            nc.sync.dma_start(utT[:], ut23_d[:, :, SEG * s : SEG * (s + 1)])
            for i in range(0, GPS, 2):
                g = GPS * s + i          # even group -> psum rows 0:64
                m7 = g // 2              # both groups share L7 block m7
                m5 = m7 // 4
                if m7 % 4 == 0 and ("b45", m5) not in emitted:
                    emitted.add(("b45", m5))
                    do_transpose(
                        f"b45_{m5}",
                        [(SF23, 2 * mm) for mm in range(4 * m5, 4 * m5 + 4)],
                        m5,
                    )
                if ("b67", m7) not in emitted:
                    emitted.add(("b67", m7))
                    do_transpose(f"b67_{m7}", [(SF23, 2 * m7 + 1)], m7)
                eps = ep.tile([128, 512], F32, tag="exp", name=f"e23_{g}")
                slA = slice(512 * i, 512 * (i + 1))
                slB = slice(512 * (i + 1), 512 * (i + 2))
                nc.tensor.matmul(
                    eps[0:64], SF23[:, 2 * m7], utT[:, 0, slA],
                    start=True, stop=False, tile_position=(0, 0),
                )
                nc.tensor.matmul(
                    eps[0:64], SF23[:, 2 * m7 + 1], utT[:, 1, slA],
                    start=False, stop=True, tile_position=(0, 0),
                )
                nc.tensor.matmul(
                    eps[64:128], SF23[:, 2 * m7], utT[:, 0, slB],
                    start=True, stop=False, tile_position=(0, 64),
                )
                nc.tensor.matmul(
                    eps[64:128], SF23[:, 2 * m7 + 1], utT[:, 1, slB],
                    start=False, stop=True, tile_position=(0, 64),
                )
                yt = y23p.tile([128, 512], F8, tag=f"y23_{m7}")
                vcopy(m7, yt[:], eps[:])
                y23.append(yt)

        # ---- consume the collective. tile_wait_until pins these to late
        # sim-time so the scheduler cannot slot them into any engine queue
        # ahead of pass F / expansion-23 work (the real AllGather completes
        # far later than the scheduler's collective model assumes). ----
        with tc.tile_wait_until(0.1):
            # whole consume-chain on gpsimd (idle during e23) except the
            # PSUM-reading SF01 copies (vector) and the PE transposes
            recv = statp.tile([B, 8, 192], BF, tag="recv")
            for k in range(8):
                nc.gpsimd.dma_start(recv[:, k], b_out[k])
            for k in range(8):
                nc.gpsimd.tensor_tensor(
                    recv[:, k], recv[:, k], msk[:, k], op=MULT
                )
            t012s = statp.tile([B, 192], BF, tag="t012s")
            nc.gpsimd.tensor_tensor(t012s[:], recv[:, 0], recv[:, 1], op=ADD)
            for k in range(2, 8):
                nc.gpsimd.tensor_tensor(
                    t012s[:], t012s[:], recv[:, k], op=ADD
                )
            stg01 = stgp.tile([B, 128], BF, tag="stg", name="tr_b01a")
            nc.gpsimd.tensor_copy(stg01[:], t012s[:, 0:128])
            tp01 = tpp.tile([128, B], BF, tag="tp_b01a", name="tp_b01a")
            nc.scalar.dma_start(tp01[:], stg01[:], transpose=True)
            nc.vector.tensor_copy(SF01[:, 0], tp01[:])
            nc.vector.tensor_copy(SF01[:, 2], tp01[:])
            for b in range(2):
                stgb = stgp.tile(
                    [B, 128], BF, tag=f"stg_b23_{b}", name=f"tr_b23_{b}"
                )
                nc.gpsimd.tensor_copy(stgb[:, 0:64], t012s[:, 128:192])
                nc.gpsimd.tensor_copy(stgb[:, 64:128], l3t[:, b ^ 1])
                stg_tiles[f"b23_{b}"] = stgb
                do_transpose(f"b23_{b}", [(SF01, 2 * b + 1)], b)

        for s in range(M // SEG):
            utT = utp.tile([128, 2, SEG], F8, tag="ut_in", name=f"ut01_{s}")
            nc.sync.dma_start(utT[:], ut01_d[:, :, SEG * s : SEG * (s + 1)])
            yo = yop.tile(
                [128, GPS // 2, 512], F8, tag=f"yout{s % 2}", name=f"yo{s}"
            )
            for i in range(0, GPS, 2):
                g = GPS * s + i
                b = g // 32
                p = g // 2
                eps = ep.tile([128, 512], F32, tag="exp", name=f"e01_{g}")
                slA = slice(512 * i, 512 * (i + 1))
                slB = slice(512 * (i + 1), 512 * (i + 2))
                nc.tensor.matmul(
                    eps[0:64], SF01[:, 2 * b], utT[:, 0, slA],
                    start=True, stop=False, tile_position=(0, 0),
                )
                nc.tensor.matmul(
                    eps[0:64], SF01[:, 2 * b + 1], utT[:, 1, slA],
                    start=False, stop=True, tile_position=(0, 0),
                )
                nc.tensor.matmul(
                    eps[64:128], SF01[:, 2 * b], utT[:, 0, slB],
                    start=True, stop=False, tile_position=(0, 64),
                )
                nc.tensor.matmul(
                    eps[64:128], SF01[:, 2 * b + 1], utT[:, 1, slB],
                    start=False, stop=True, tile_position=(0, 64),
                )
                nc.vector.tensor_tensor(
                    yo[:, i // 2], eps[:], y23[p][:], op=ADD
                )
            p0 = (GPS * s) // 2
            for h in range(2):
                nc.scalar.dma_start(
                    corr_d[:, p0 : p0 + GPS // 2, h, :],
                    yo[64 * h : 64 * (h + 1)],
                )

    nc.compile()
    return nc


def _pack_inputs(x, diag, u):
    """Build per-core input maps. x (B,N,1) f32, u (DEPTH,N,R) f32."""
    in_maps = []
    x2 = np.asarray(x).reshape(B, N)
    u3 = np.asarray(u)
    ident = np.eye(B, dtype=BF16)
    for c in range(NCORES):
        base = c * M
        xsl = x2[:, base : base + M]                      # (B, M)
        usl = u3[:, base : base + M, :] * USCALE          # (8, M, 64)
        # xs[p, j, ko, b] = x[b, 256j + 128ko + p]
        xs = np.ascontiguousarray(
            xsl.reshape(B, CP, 2, 128).transpose(3, 1, 2, 0)
        ).astype(FP8)
        # uc/uf[p, j, ko, l*64+r] = u[l(+4), 256j + 128ko + p, r]
        u5 = usl.reshape(8, CP, 2, 128, R)
        uc = np.ascontiguousarray(
            u5[0:4].transpose(3, 1, 2, 0, 4).reshape(128, CP, 2, 256)
        ).astype(FP8)
        uf = np.ascontiguousarray(
            u5[4:8].transpose(3, 1, 2, 0, 4).reshape(128, CP, 2, 256)
        ).astype(FP8)
        ut4 = usl.transpose(0, 2, 1).reshape(4, 128, M)
        ut23 = np.ascontiguousarray(ut4[2:4].transpose(1, 0, 2)).astype(FP8)
        ut01 = np.ascontiguousarray(ut4[0:2].transpose(1, 0, 2)).astype(FP8)
        # receive mask: msk[:, k, 64l:64l+64] = 1 iff src core k is in
        # this core c's level-l sibling block
        msk = np.zeros((B, 8, 192), dtype=BF16)
        for k in range(8):
            if (k // 4) == ((c // 4) ^ 1):
                msk[:, k, 0:64] = 1.0
            if (k // 2) == ((c // 2) ^ 1):
                msk[:, k, 64:128] = 1.0
            if k == c ^ 1:
                msk[:, k, 128:192] = 1.0
        in_maps.append(
            {
                "xs": xs,
                "uc": uc,
                "uf": uf,
                "ut23": ut23,
                "ut01": ut01,
                "msk": msk,
                "ident": ident,
            }
        )
    return in_maps


last_results = None


def kernel(x, diag, u):
    global last_results
    from concourse.bass_utils import run_bass_kernel_spmd

    if "nc" not in _cached:
        _cached["nc"] = _build_bass()
    nc = _cached["nc"]

    in_maps = _pack_inputs(x, diag, u)
    res = run_bass_kernel_spmd(nc, in_maps, core_ids=list(range(NCORES)))
    last_results = res

    x2 = np.asarray(x, dtype=np.float32).reshape(B, N)
    d2 = np.asarray(diag, dtype=np.float32).reshape(1, N)
    y = d2 * x2
    inv = 1.0 / (USCALE * USCALE)
    for c in range(NCORES):
        corr = res.results[c]["corr"].astype(np.float32).reshape(B, M)
        y[:, c * M : (c + 1) * M] += corr * inv
    return y.reshape(B, N, 1).astype(np.float32)
